# revision 23
# baseline (speedup 1.0000x reference)
"""MoE routing gate kernel for Trainium2 (8 NeuronCores, data-parallel).

Problem (hardcoded): x [4, 4096, 2048] f32, w_gate [64, 2048] f32,
expert_bias [64] f32 (zeros per spec).
  gate_logits = x @ w_gate.T          # [B, S, 64]
  gate_weights = sigmoid(gate_logits)
  topk_vals, topk_idx = top_k(gate_logits + bias, k=8)
  topk_weights = gather(gate_weights, topk_idx); normalize
Returns (topk_weights [4,4096,8] f32, topk_indices [4,4096,8] int32).

Strategy: shard the 16384 tokens across 8 cores (2048 each). The
kernel is HBM-bandwidth bound (16.8 MB/core, ~45 us at ~370 GB/s), so
the matmul must run faster than the stream. fp32 matmuls cost 4 PE
cycles/row; instead each fp32 value is split host-side into an exact
fp16 hi+lo pair (scaled by 2^11 to dodge fp16 denormals), which the
PE multiplies exactly (11x11-bit products) into fp32 PSUM at 1
cycle/row. The stationary operand packs [wh | wl] (128 cols), so TWO
moving passes (xh, xl) accumulate all four cross terms:
  psum[0:64]   += xh@wh + xl@wh
  psum[64:128] += xh@wl + xl@wl
i.e. 2 cycles/row total for a numerically fp32-grade product (logit
error ~4e-7 std; top-8 selection margins verified host-side).

The two PSUM halves are merged AND transposed token-major in one PE
matmul per 128-token tile: lhsT = logits[128 exp-halves, 128 tok]
(fp32 stationary), rhs = stacked identity [I64; I64] -> psum
[128 tok, 64 exp] = hi + lo transposed. Then per tile the DVE max8 /
max_index8 ops give top-8 values+indices, ACT sigmoid (with the 2^-22
descale folded in), DVE sum/reciprocal/scalar-mul normalize.

The last 512-token group is processed as two 256-token halves with
fine-grained trailing DMA chunks so only ~4 us of work remains after
the final HBM byte lands. Expert bias is zeros per the problem spec
(a numpy fallback guards the general case).
"""

import numpy as np

_B, _S, _D, _E = 4, 4096, 2048, 64
_K = 8
_NCORES = 8
_TOK = _B * _S              # 16384 tokens
_TC = _TOK // _NCORES       # 2048 tokens per core
_NG = 4                     # token groups of 512 per core
_GT = 512                   # tokens per group (PSUM bank)
_NKC = _D // 128            # 16 contraction chunks
_SCALE = float(2.0 ** 11)   # per-operand scale (fp16 denormal guard)
_DESCALE = float(2.0 ** -22)

_prog_cache = {}


def _ensure_path():
    import sys
    for p in ("/opt/trn_rl_repo",):
        if p not in sys.path:
            sys.path.insert(0, p)


def _build_program():
    """Per-core Bass/Tile program (SPMD: same program, different data)."""
    _ensure_path()
    import concourse.bass as bass
    import concourse.tile as tile
    from concourse import bacc, mybir

    nc = bacc.Bacc("TRN2", target_bir_lowering=False, debug=False,
                   num_devices=_NCORES)

    f32 = mybir.dt.float32
    u32 = mybir.dt.uint32
    f16 = mybir.dt.float16

    # DRAM I/O (per core). x layout: [g, dp, k, hl, tau]: for token group
    # g, partition dp (d % 128), contraction chunk k (d // 128), hl=0 the
    # fp16 hi part / hl=1 the lo part, tau token-in-group. Each group is
    # one fully-contiguous-per-partition 32 KiB block.
    xg = nc.dram_tensor("xg", [_NG, 128, _NKC, 2, _GT], f16,
                        kind="ExternalInput")
    # wt[dp, k, j]: j<64 -> hi(w[e=j]), j>=64 -> lo(w[e=j-64]).
    wt = nc.dram_tensor("wt", [128, _NKC, 128], f16, kind="ExternalInput")
    # Stacked identity [I64; I64] merges the hi/lo PSUM halves during the
    # token-major transpose matmul.
    ident2 = nc.dram_tensor("ident2", [128, _E], f32, kind="ExternalInput")
    out_w = nc.dram_tensor("out_w", [128, _NG, _NG, _K], f32,
                           kind="ExternalOutput")
    out_i = nc.dram_tensor("out_i", [128, _NG, _NG, _K], u32,
                           kind="ExternalOutput")

    # k-chunk split per group's DMA: fine-grained first loads so the PE
    # starts early; coarse in the middle for DMA efficiency; fine again
    # at the very end so almost nothing waits on the last byte.
    subchunks = ((1, 3, 4, 8), (8, 8), (8, 8), (4, 4, 4, 2, 1, 1))

    with tile.TileContext(nc) as tc:
        with (
            tc.tile_pool(name="xpool", bufs=3) as xpool,
            tc.tile_pool(name="wpool", bufs=1) as wpool,
            tc.tile_pool(name="psA", bufs=3, space=bass.MemorySpace.PSUM) as psA,
            tc.tile_pool(name="psB", bufs=2, space=bass.MemorySpace.PSUM) as psB,
            tc.tile_pool(name="lpool", bufs=2) as lpool,
            tc.tile_pool(name="opool", bufs=2) as opool,
            tc.tile_pool(name="tpool", bufs=4) as tpool,
        ):
            # PE warm-up: ~4 us of dummy matmuls on a zeroed tile flips
            # the HAM clock gate to 8/8 (2.4 GHz) before the real stream
            # arrives; a cold PE (1.2 GHz) cannot keep up with HBM.
            warm = wpool.tile([128, _GT], f16)
            nc.gpsimd.memset(warm[:], 0)
            wps = psA.tile([128, _GT], f32, tag="ps0")
            for _ in range(9):
                nc.tensor.matmul(wps[:], warm[:, 0:128], warm[:],
                                 start=True, stop=True)

            # x rides the sync ring exclusively; weights + identity ride
            # the scalar ring so both streams start in parallel and the
            # small weight descriptors never stall the bulk x FIFO.
            # x rides the sync ring exclusively; weights + identity ride
            # the scalar ring. The bulk weight slice then arrives a few
            # us late (minority SDMA share), idling the PE once early —
            # harmless in the DMA-bound phase with 3 groups buffered —
            # after which the PE runs warm and uninterrupted to the end.
            # (Measured better than consumption-ordered loads on one
            # ring, which spreads small PE idles across every group
            # boundary and makes the HAM clock gate oscillate.)
            xt0 = xpool.tile([128, _NKC, 2, _GT], f16, tag="xg")
            nc.sync.dma_start(xt0[:, 0:1, :, :], xg[0][:, 0:1, :, :])
            wt0_sb = wpool.tile([128, 128], f16)
            nc.scalar.dma_start(wt0_sb[:], wt[:, 0, :])
            wtR_sb = wpool.tile([128, _NKC - 1, 128], f16)
            nc.scalar.dma_start(wtR_sb[:], wt[:, 1:, :])
            id_sb = wpool.tile([128, _E], f32)
            nc.scalar.dma_start(id_sb[:], ident2[:])

            def wt_k(k):
                return wt0_sb[:] if k == 0 else wtR_sb[:, k - 1, :]

            def postprocess(lg2, ig, wg, jg, nj):
                """top-8 + sigmoid(descale) + normalize, nj 128-tok tiles."""
                for j in range(nj):
                    logit = lg2[:, j, :]
                    vals = tpool.tile([128, _K], f32, tag="vals")
                    nc.vector.max(vals[:], logit)
                    nc.vector.max_index(ig[:, jg + j, :], vals[:], logit)

                    sig = tpool.tile([128, _K], f32, tag="sig")
                    # (accum_out fusion measured WORSE: it adds a ~220ns
                    # ACTIVATION_READ_ACCUMULATOR op per tile.)
                    nc.scalar.activation(
                        sig[:], vals[:], mybir.ActivationFunctionType.Sigmoid,
                        scale=_DESCALE,
                    )
                    ssum = tpool.tile([128, 1], f32, tag="ssum")
                    nc.vector.reduce_sum(
                        ssum[:], sig[:], axis=mybir.AxisListType.X,
                    )
                    rsum = tpool.tile([128, 1], f32, tag="rsum")
                    nc.vector.reciprocal(rsum[:], ssum[:])
                    nc.vector.tensor_scalar_mul(wg[:, jg + j, :], sig[:], rsum[:])

            def finish_block(ps, g, toff, ntok, ig, wg):
                """psum [128 exp-halves, ntok] -> topk results in wg/ig."""
                nj = ntok // 128
                lg = lpool.tile([128, _GT], f32, tag="lg")
                nc.scalar.copy(lg[:, :ntok], ps[:])
                ps2 = psB.tile([128, _NG, _E], f32, tag="ps2")
                for j in range(nj):
                    # [128 tok, 64 exp] = lg[:, j-slice].T @ [I64; I64]
                    # (merges hi+lo halves while transposing).
                    nc.tensor.matmul(
                        ps2[:, j, :], lg[:, bass.ts(j, 128)], id_sb[:],
                        start=True, stop=True,
                    )
                # DVE max8/max_index read the transposed logits directly
                # from PSUM — no second ACT copy on the critical tail.
                postprocess(ps2, ig, wg, toff // 128, nj)

            # All 16 token-tiles' results accumulate in SBUF; a single
            # out-DMA pair at the very end keeps the tiny-descriptor
            # output writes OUT of the x-stream FIFO (they stall the ring
            # ~1.5 us each when interleaved).
            wg_all = opool.tile([128, _NG, _NG, _K], f32)
            ig_all = opool.tile([128, _NG, _NG, _K], u32)

            def finish_group(g, splits, pss):
                """Merge/transpose/top-k for one finished group."""
                for (toff, ntok), ps in zip(splits, pss):
                    finish_block(ps, g, toff, ntok, ig_all[:, g], wg_all[:, g])

            pending = None
            for g in range(_NG):
                xt = xt0 if g == 0 else xpool.tile([128, _NKC, 2, _GT], f16,
                                                   tag="xg")
                k0 = 1 if g == 0 else 0
                for nk in subchunks[g]:
                    nk = min(nk, _NKC - k0)
                    nc.sync.dma_start(
                        xt[:, k0:k0 + nk, :, :],
                        xg[g][:, k0:k0 + nk, :, :],
                    )
                    k0 += nk

                # Last group: two 256-token accumulators so the final
                # top-k tail is halved (first half overlaps second's MMs).
                splits = ((0, _GT),) if g < _NG - 1 else ((0, 256), (256, 256))
                pss = []
                for toff, ntok in splits:
                    ps = psA.tile([128, ntok], f32, tag=f"ps{len(pss)}")
                    pss.append(ps)
                for k in range(_NKC):
                    for (toff, ntok), ps in zip(splits, pss):
                        for hl in range(2):
                            nc.tensor.matmul(
                                ps[:], wt_k(k),
                                xt[:, k, hl, toff:toff + ntok],
                                start=(k == 0 and hl == 0),
                                stop=(k == _NKC - 1 and hl == 1),
                            )

                # Software pipeline: the previous group's merge/top-k is
                # emitted AFTER this group's matmuls, so the PE never
                # stalls on the ACT psum-copy between groups.
                if pending is not None:
                    finish_group(*pending)
                    if pending[0] == _NG - 2:
                        # Groups 0..NG-2 store mid-stream on the (idle)
                        # scalar ring; only the last group's small outs
                        # remain on the critical tail.
                        nc.scalar.dma_start(out_w[:, :_NG - 1],
                                            wg_all[:, :_NG - 1])
                        nc.scalar.dma_start(out_i[:, :_NG - 1],
                                            ig_all[:, :_NG - 1])
                pending = (g, splits, pss)
            finish_group(*pending)
            # w-out on the sync ring, i-out on the scalar ring so the two
            # final descriptor generations run in parallel.
            nc.sync.dma_start(out_w[:, _NG - 1:], wg_all[:, _NG - 1:])
            nc.scalar.dma_start(out_i[:, _NG - 1:], ig_all[:, _NG - 1:])

    _hoist_prebarrier_dmas(nc, mybir)
    nc.compile()
    return nc


def _hoist_prebarrier_dmas(nc, mybir):
    """Move the opening (waitless) input DMAs from the tile block to the
    main block, BEFORE each engine's entry-barrier Drain. The exec-time
    clock starts when instruction queues load (~5.5 us), but descriptor
    generation otherwise waits for the all-engine barrier (~7 us) and,
    on ACT, for the 1.3 us activation-table load. Hoisting starts the
    HBM stream ~2 us earlier. The hoisted DMAs touch no const SBUF and
    keep their completion-semaphore updates, so downstream waits hold.
    """
    f = nc.m.functions[0]
    main_blk, tile_blk = f.blocks[0], f.blocks[1]
    hoist = []
    for ins in list(tile_blk.instructions[:8]):
        if type(ins).__name__ == "InstDMACopy" and "wait:" not in ins.concise():
            hoist.append(ins)
    if not hoist:
        return
    for ins in hoist:
        tile_blk.instructions.remove(ins)
    # Insert each before its engine's first instruction (the barrier
    # Drain) in main; per-engine stream order is list order filtered by
    # engine, so position 0 is safe for every engine.
    for ins in reversed(hoist):
        main_blk.instructions.insert(1, ins)


def _get_program():
    if "fp16x2" not in _prog_cache:
        _prog_cache["fp16x2"] = _build_program()
    return _prog_cache["fp16x2"]


def _pack_inputs(x, w_gate):
    """Host-side layout transform + fp16 hi/lo split (scaled by 2^11)."""
    x2 = np.ascontiguousarray(x, dtype=np.float32).reshape(_TOK, _D)
    xs = x2 * np.float32(_SCALE)
    xh = xs.astype(np.float16)
    xl = (xs - xh.astype(np.float32)).astype(np.float16)

    w = np.asarray(w_gate, dtype=np.float32) * np.float32(_SCALE)
    wh = w.astype(np.float16)
    wl = (w - wh.astype(np.float32)).astype(np.float16)
    # wt[dp, k, j]: j<64 hi, j>=64 lo; w[e, k*128+dp].
    wt = np.concatenate(
        [
            wh.T.reshape(_NKC, 128, _E).transpose(1, 0, 2),
            wl.T.reshape(_NKC, 128, _E).transpose(1, 0, 2),
        ],
        axis=2,
    )
    wt = np.ascontiguousarray(wt)                  # [128, 16, 128] f16

    ident2 = np.tile(np.eye(_E, dtype=np.float32), (2, 1))  # [128, 64]

    in_maps = []
    for c in range(_NCORES):
        sl = slice(c * _TC, (c + 1) * _TC)
        # [g, tau, k, dp] -> [g, dp, k, hl, tau]
        xgh = xh[sl].reshape(_NG, _GT, _NKC, 128).transpose(0, 3, 2, 1)
        xgl = xl[sl].reshape(_NG, _GT, _NKC, 128).transpose(0, 3, 2, 1)
        xgc = np.ascontiguousarray(
            np.stack([xgh, xgl], axis=3)           # [g, dp, k, hl, tau]
        )
        in_maps.append({"xg": xgc, "wt": wt, "ident2": ident2})
    return in_maps


def _unpack_outputs(results):
    w_parts, i_parts = [], []
    for r in results:
        # [128 tau, 4 g, 4 j, 8] -> token (4g+j)*128+tau -> [2048, 8]
        w_parts.append(
            r["out_w"].reshape(128, _NG * _NG, _K).transpose(1, 0, 2).reshape(_TC, _K)
        )
        i_parts.append(
            r["out_i"].reshape(128, _NG * _NG, _K).transpose(1, 0, 2).reshape(_TC, _K)
        )
    weights = np.concatenate(w_parts, axis=0).reshape(_B, _S, _K)
    indices = (
        np.concatenate(i_parts, axis=0).astype(np.int32).reshape(_B, _S, _K)
    )
    return weights, indices


def _numpy_reference(x, w_gate, expert_bias):
    """Exact fallback for the (unspecced) nonzero-bias case."""
    x2 = np.asarray(x, dtype=np.float32).reshape(_TOK, _D)
    logits = x2 @ np.asarray(w_gate, dtype=np.float32).T
    gw = 1.0 / (1.0 + np.exp(-logits))
    biased = logits + np.asarray(expert_bias, dtype=np.float32)
    idx = np.argsort(-biased, axis=-1, kind="stable")[:, :_K].astype(np.int32)
    tw = np.take_along_axis(gw, idx, axis=-1)
    tw = tw / tw.sum(axis=-1, keepdims=True)
    return (
        tw.reshape(_B, _S, _K).astype(np.float32),
        idx.reshape(_B, _S, _K).astype(np.int32),
    )


def _run(x, w_gate, expert_bias, trace=False, mode=None, trace_kwargs=None):
    _ensure_path()
    from concourse.bass_utils import run_bass_kernel_spmd

    nc = _get_program()
    in_maps = _pack_inputs(x, w_gate)
    res = run_bass_kernel_spmd(
        nc, in_maps, list(range(_NCORES)), trace=trace,
        **(trace_kwargs or {}),
    )
    weights, indices = _unpack_outputs(res.results)
    return (weights, indices), res


def kernel(x, w_gate, expert_bias):
    x = np.asarray(x)
    w_gate = np.asarray(w_gate)
    expert_bias = np.asarray(expert_bias)
    assert x.shape == (_B, _S, _D), x.shape
    assert w_gate.shape == (_E, _D), w_gate.shape
    if np.any(expert_bias):
        # Spec pins expert_bias to zeros; keep a correct host path anyway.
        return _numpy_reference(x, w_gate, expert_bias)
    try:
        (weights, indices), _ = _run(x, w_gate, expert_bias)
    except Exception:
        # Transient NRT device wedges have been observed on a first
        # execution; one retry has always recovered.
        import time
        time.sleep(10)
        (weights, indices), _ = _run(x, w_gate, expert_bias)
    return weights, indices


# revision 25
# speedup vs baseline: 1.1697x; 1.1697x over previous
"""MoE routing gate kernel for Trainium2 (8 NeuronCores, data-parallel).

Problem (hardcoded): x [4, 4096, 2048] f32, w_gate [64, 2048] f32,
expert_bias [64] f32 (zeros per spec).
  gate_logits = x @ w_gate.T          # [B, S, 64]
  gate_weights = sigmoid(gate_logits)
  topk_vals, topk_idx = top_k(gate_logits + bias, k=8)
  topk_weights = gather(gate_weights, topk_idx); normalize
Returns (topk_weights [4,4096,8] f32, topk_indices [4,4096,8] int32).

Strategy: shard the 16384 tokens across 8 cores (2048 each). The
kernel is HBM-bandwidth bound (16.8 MB/core, ~45 us at ~370 GB/s), so
the matmul must run faster than the stream. fp32 matmuls cost 4 PE
cycles/row; instead each fp32 value is split host-side into an exact
fp16 hi+lo pair (scaled by 2^11 to dodge fp16 denormals), which the
PE multiplies exactly (11x11-bit products) into fp32 PSUM at 1
cycle/row. The stationary operand packs [wh | wl] (128 cols), so TWO
moving passes (xh, xl) accumulate all four cross terms:
  psum[0:64]   += xh@wh + xl@wh
  psum[64:128] += xh@wl + xl@wl
i.e. 2 cycles/row total for a numerically fp32-grade product (logit
error ~4e-7 std; top-8 selection margins verified host-side).

The two PSUM halves are merged AND transposed token-major in one PE
matmul per 128-token tile: lhsT = logits[128 exp-halves, 128 tok]
(fp32 stationary), rhs = stacked identity [I64; I64] -> psum
[128 tok, 64 exp] = hi + lo transposed. Then per tile the DVE max8 /
max_index8 ops give top-8 values+indices, ACT sigmoid (with the 2^-22
descale folded in), DVE sum/reciprocal/scalar-mul normalize.

The last 512-token group is processed as two 256-token halves with
fine-grained trailing DMA chunks so only ~4 us of work remains after
the final HBM byte lands. Expert bias is zeros per the problem spec
(a numpy fallback guards the general case).
"""

import numpy as np

_B, _S, _D, _E = 4, 4096, 2048, 64
_K = 8
_NCORES = 8
_TOK = _B * _S              # 16384 tokens
_TC = _TOK // _NCORES       # 2048 tokens per core
_NG = 4                     # token groups of 512 per core
_GT = 512                   # tokens per group (PSUM bank)
_NKC = _D // 128            # 16 contraction chunks
_SCALE = float(2.0 ** 11)   # per-operand scale (fp16 denormal guard)
_DESCALE = float(2.0 ** -22)

_prog_cache = {}


def _ensure_path():
    import sys
    for p in ("/opt/trn_rl_repo",):
        if p not in sys.path:
            sys.path.insert(0, p)


def _build_program():
    """Per-core Bass/Tile program (SPMD: same program, different data)."""
    _ensure_path()
    import concourse.bass as bass
    import concourse.tile as tile
    from concourse import bacc, mybir

    nc = bacc.Bacc("TRN2", target_bir_lowering=False, debug=False,
                   num_devices=_NCORES)

    f32 = mybir.dt.float32
    u32 = mybir.dt.uint32
    f16 = mybir.dt.float16

    # DRAM I/O (per core). x layout: [g, dp, k, hl, tau]: for token group
    # g, partition dp (d % 128), contraction chunk k (d // 128), hl=0 the
    # fp16 hi part / hl=1 the lo part, tau token-in-group. Each group is
    # one fully-contiguous-per-partition 32 KiB block.
    xg = nc.dram_tensor("xg", [_NG, 128, _NKC, 2, _GT], f16,
                        kind="ExternalInput")
    # wt[dp, k, j]: j<64 -> hi(w[e=j]), j>=64 -> lo(w[e=j-64]).
    wt = nc.dram_tensor("wt", [128, _NKC, 128], f16, kind="ExternalInput")
    # Stacked identity [I64; I64] merges the hi/lo PSUM halves during the
    # token-major transpose matmul.
    ident2 = nc.dram_tensor("ident2", [128, _E], f32, kind="ExternalInput")
    out_w = nc.dram_tensor("out_w", [128, _NG, _NG, _K], f32,
                           kind="ExternalOutput")
    out_i = nc.dram_tensor("out_i", [128, _NG, _NG, _K], u32,
                           kind="ExternalOutput")

    # k-chunk split per group's DMA: fine-grained first loads so the PE
    # starts early; coarse in the middle for DMA efficiency; fine again
    # at the very end so almost nothing waits on the last byte.
    subchunks = ((1, 3, 4, 8), (8, 8), (8, 8), (4, 4, 4, 2, 1, 1))

    with tile.TileContext(nc) as tc:
        with (
            tc.tile_pool(name="xpool", bufs=3) as xpool,
            tc.tile_pool(name="wpool", bufs=1) as wpool,
            tc.tile_pool(name="psA", bufs=3, space=bass.MemorySpace.PSUM) as psA,
            tc.tile_pool(name="psB", bufs=2, space=bass.MemorySpace.PSUM) as psB,
            tc.tile_pool(name="lpool", bufs=2) as lpool,
            tc.tile_pool(name="opool", bufs=2) as opool,
            tc.tile_pool(name="tpool", bufs=4) as tpool,
        ):
            # PE warm-up: ~4 us of dummy matmuls on a zeroed tile flips
            # the HAM clock gate to 8/8 (2.4 GHz) before the real stream
            # arrives; a cold PE (1.2 GHz) cannot keep up with HBM.
            warm = wpool.tile([128, _GT], f16)
            nc.gpsimd.memset(warm[:], 0)
            wps = psA.tile([128, _GT], f32, tag="ps0")
            for _ in range(9):
                nc.tensor.matmul(wps[:], warm[:, 0:128], warm[:],
                                 start=True, stop=True)

            # x rides the sync ring exclusively; weights + identity ride
            # the scalar ring so both streams start in parallel and the
            # small weight descriptors never stall the bulk x FIFO.
            # x rides the sync ring exclusively; weights + identity ride
            # the scalar ring. The bulk weight slice then arrives a few
            # us late (minority SDMA share), idling the PE once early —
            # harmless in the DMA-bound phase with 3 groups buffered —
            # after which the PE runs warm and uninterrupted to the end.
            # (Measured better than consumption-ordered loads on one
            # ring, which spreads small PE idles across every group
            # boundary and makes the HAM clock gate oscillate.)
            xt0 = xpool.tile([128, _NKC, 2, _GT], f16, tag="xg")
            nc.sync.dma_start(xt0[:, 0:1, :, :], xg[0][:, 0:1, :, :])
            wt0_sb = wpool.tile([128, 128], f16)
            nc.scalar.dma_start(wt0_sb[:], wt[:, 0, :])
            wtR_sb = wpool.tile([128, _NKC - 1, 128], f16)
            nc.scalar.dma_start(wtR_sb[:], wt[:, 1:, :])
            id_sb = wpool.tile([128, _E], f32)
            nc.scalar.dma_start(id_sb[:], ident2[:])

            def wt_k(k):
                return wt0_sb[:] if k == 0 else wtR_sb[:, k - 1, :]

            def postprocess(lg2, ig, wg, jg, nj):
                """top-8 + sigmoid(descale) + normalize, nj 128-tok tiles."""
                for j in range(nj):
                    logit = lg2[:, j, :]
                    vals = tpool.tile([128, _K], f32, tag="vals")
                    nc.vector.max(vals[:], logit)
                    nc.vector.max_index(ig[:, jg + j, :], vals[:], logit)

                    sig = tpool.tile([128, _K], f32, tag="sig")
                    # (accum_out fusion measured WORSE: it adds a ~220ns
                    # ACTIVATION_READ_ACCUMULATOR op per tile.)
                    nc.scalar.activation(
                        sig[:], vals[:], mybir.ActivationFunctionType.Sigmoid,
                        scale=_DESCALE,
                    )
                    ssum = tpool.tile([128, 1], f32, tag="ssum")
                    nc.vector.reduce_sum(
                        ssum[:], sig[:], axis=mybir.AxisListType.X,
                    )
                    rsum = tpool.tile([128, 1], f32, tag="rsum")
                    nc.vector.reciprocal(rsum[:], ssum[:])
                    nc.vector.tensor_scalar_mul(wg[:, jg + j, :], sig[:], rsum[:])

            def finish_block(ps, g, toff, ntok, ig, wg):
                """psum [128 exp-halves, ntok] -> topk results in wg/ig."""
                nj = ntok // 128
                lg = lpool.tile([128, _GT], f32, tag="lg")
                nc.scalar.copy(lg[:, :ntok], ps[:])
                ps2 = psB.tile([128, _NG, _E], f32, tag="ps2")
                for j in range(nj):
                    # [128 tok, 64 exp] = lg[:, j-slice].T @ [I64; I64]
                    # (merges hi+lo halves while transposing).
                    nc.tensor.matmul(
                        ps2[:, j, :], lg[:, bass.ts(j, 128)], id_sb[:],
                        start=True, stop=True,
                    )
                lg2 = lpool.tile([128, _NG, _E], f32, tag="lg2")
                nc.scalar.copy(lg2[:, :nj, :], ps2[:, :nj, :])
                postprocess(lg2, ig, wg, toff // 128, nj)

            # All 16 token-tiles' results accumulate in SBUF; a single
            # out-DMA pair at the very end keeps the tiny-descriptor
            # output writes OUT of the x-stream FIFO (they stall the ring
            # ~1.5 us each when interleaved).
            wg_all = opool.tile([128, _NG, _NG, _K], f32)
            ig_all = opool.tile([128, _NG, _NG, _K], u32)

            def finish_group(g, splits, pss):
                """Merge/transpose/top-k for one finished group."""
                for (toff, ntok), ps in zip(splits, pss):
                    finish_block(ps, g, toff, ntok, ig_all[:, g], wg_all[:, g])

            pending = None
            for g in range(_NG):
                xt = xt0 if g == 0 else xpool.tile([128, _NKC, 2, _GT], f16,
                                                   tag="xg")
                k0 = 1 if g == 0 else 0
                for nk in subchunks[g]:
                    nk = min(nk, _NKC - k0)
                    nc.sync.dma_start(
                        xt[:, k0:k0 + nk, :, :],
                        xg[g][:, k0:k0 + nk, :, :],
                    )
                    k0 += nk

                # Last group: two 256-token accumulators so the final
                # top-k tail is halved (first half overlaps second's MMs).
                splits = ((0, _GT),) if g < _NG - 1 else ((0, 256), (256, 256))
                pss = []
                for toff, ntok in splits:
                    ps = psA.tile([128, ntok], f32, tag=f"ps{len(pss)}")
                    pss.append(ps)
                for k in range(_NKC):
                    for (toff, ntok), ps in zip(splits, pss):
                        for hl in range(2):
                            nc.tensor.matmul(
                                ps[:], wt_k(k),
                                xt[:, k, hl, toff:toff + ntok],
                                start=(k == 0 and hl == 0),
                                stop=(k == _NKC - 1 and hl == 1),
                            )

                # Software pipeline: the previous group's merge/top-k is
                # emitted AFTER this group's matmuls, so the PE never
                # stalls on the ACT psum-copy between groups.
                if pending is not None:
                    finish_group(*pending)
                    if pending[0] == _NG - 2:
                        # Groups 0..NG-2 store mid-stream on the (idle)
                        # scalar ring; only the last group's small outs
                        # remain on the critical tail.
                        nc.scalar.dma_start(out_w[:, :_NG - 1],
                                            wg_all[:, :_NG - 1])
                        nc.scalar.dma_start(out_i[:, :_NG - 1],
                                            ig_all[:, :_NG - 1])
                pending = (g, splits, pss)
            finish_group(*pending)
            # w-out on the sync ring, i-out on the scalar ring so the two
            # final descriptor generations run in parallel.
            nc.sync.dma_start(out_w[:, _NG - 1:], wg_all[:, _NG - 1:])
            nc.scalar.dma_start(out_i[:, _NG - 1:], ig_all[:, _NG - 1:])

    # NOTE: hoisting the opening DMAs before the entry barrier (into the
    # main block) was tried and measured WORSE — the early bulk DMAs
    # contend with the engines' instruction-queue refill DMAs at boot.
    nc.compile()
    return nc


def _get_program():
    if "fp16x2" not in _prog_cache:
        _prog_cache["fp16x2"] = _build_program()
    return _prog_cache["fp16x2"]


def _pack_inputs(x, w_gate):
    """Host-side layout transform + fp16 hi/lo split (scaled by 2^11)."""
    x2 = np.ascontiguousarray(x, dtype=np.float32).reshape(_TOK, _D)
    xs = x2 * np.float32(_SCALE)
    xh = xs.astype(np.float16)
    xl = (xs - xh.astype(np.float32)).astype(np.float16)

    w = np.asarray(w_gate, dtype=np.float32) * np.float32(_SCALE)
    wh = w.astype(np.float16)
    wl = (w - wh.astype(np.float32)).astype(np.float16)
    # wt[dp, k, j]: j<64 hi, j>=64 lo; w[e, k*128+dp].
    wt = np.concatenate(
        [
            wh.T.reshape(_NKC, 128, _E).transpose(1, 0, 2),
            wl.T.reshape(_NKC, 128, _E).transpose(1, 0, 2),
        ],
        axis=2,
    )
    wt = np.ascontiguousarray(wt)                  # [128, 16, 128] f16

    ident2 = np.tile(np.eye(_E, dtype=np.float32), (2, 1))  # [128, 64]

    in_maps = []
    for c in range(_NCORES):
        sl = slice(c * _TC, (c + 1) * _TC)
        # [g, tau, k, dp] -> [g, dp, k, hl, tau]
        xgh = xh[sl].reshape(_NG, _GT, _NKC, 128).transpose(0, 3, 2, 1)
        xgl = xl[sl].reshape(_NG, _GT, _NKC, 128).transpose(0, 3, 2, 1)
        xgc = np.ascontiguousarray(
            np.stack([xgh, xgl], axis=3)           # [g, dp, k, hl, tau]
        )
        in_maps.append({"xg": xgc, "wt": wt, "ident2": ident2})
    return in_maps


def _unpack_outputs(results):
    w_parts, i_parts = [], []
    for r in results:
        # [128 tau, 4 g, 4 j, 8] -> token (4g+j)*128+tau -> [2048, 8]
        w_parts.append(
            r["out_w"].reshape(128, _NG * _NG, _K).transpose(1, 0, 2).reshape(_TC, _K)
        )
        i_parts.append(
            r["out_i"].reshape(128, _NG * _NG, _K).transpose(1, 0, 2).reshape(_TC, _K)
        )
    weights = np.concatenate(w_parts, axis=0).reshape(_B, _S, _K)
    indices = (
        np.concatenate(i_parts, axis=0).astype(np.int32).reshape(_B, _S, _K)
    )
    return weights, indices


def _numpy_reference(x, w_gate, expert_bias):
    """Exact fallback for the (unspecced) nonzero-bias case."""
    x2 = np.asarray(x, dtype=np.float32).reshape(_TOK, _D)
    logits = x2 @ np.asarray(w_gate, dtype=np.float32).T
    gw = 1.0 / (1.0 + np.exp(-logits))
    biased = logits + np.asarray(expert_bias, dtype=np.float32)
    idx = np.argsort(-biased, axis=-1, kind="stable")[:, :_K].astype(np.int32)
    tw = np.take_along_axis(gw, idx, axis=-1)
    tw = tw / tw.sum(axis=-1, keepdims=True)
    return (
        tw.reshape(_B, _S, _K).astype(np.float32),
        idx.reshape(_B, _S, _K).astype(np.int32),
    )


def _run(x, w_gate, expert_bias, trace=False, mode=None, trace_kwargs=None):
    _ensure_path()
    from concourse.bass_utils import run_bass_kernel_spmd

    nc = _get_program()
    in_maps = _pack_inputs(x, w_gate)
    res = run_bass_kernel_spmd(
        nc, in_maps, list(range(_NCORES)), trace=trace,
        **(trace_kwargs or {}),
    )
    weights, indices = _unpack_outputs(res.results)
    return (weights, indices), res


def kernel(x, w_gate, expert_bias):
    x = np.asarray(x)
    w_gate = np.asarray(w_gate)
    expert_bias = np.asarray(expert_bias)
    assert x.shape == (_B, _S, _D), x.shape
    assert w_gate.shape == (_E, _D), w_gate.shape
    if np.any(expert_bias):
        # Spec pins expert_bias to zeros; keep a correct host path anyway.
        return _numpy_reference(x, w_gate, expert_bias)
    try:
        (weights, indices), _ = _run(x, w_gate, expert_bias)
    except Exception:
        # Transient NRT device wedges have been observed on a first
        # execution; one retry has always recovered.
        import time
        time.sleep(10)
        (weights, indices), _ = _run(x, w_gate, expert_bias)
    return weights, indices
